# revision 6
# baseline (speedup 1.0000x reference)
"""GQA causal-attention prefill kernel for Trainium2, tensor-parallel over 8 NeuronCores.

v2: all-bf16 datapath, fused single-stream schedule.

Sharding: head-parallel. Core c gets q heads [4c, 4c+4), kv head c, and the
matching wo slice; each core computes a full-shape partial output
o_part = attn(heads of c) @ wo_c and the host sums the 8 partials.

Per-core schedule (one PE stream, no phase barriers):
  for b, c:   PROJ(b,c) k-sweep -> OPROJ(prev chunk) -> ATTN(b,c)
The o-projection of the previous chunk fills the PE while the current
chunk's rope evictions run on DVE, so the PE never waits on the rope tail.

Layouts:
  - xT [D, B*T] bf16; projections contract D on the partition dim:
    qT/kT/vT[h] = w[h].T @ xT -> [H=128, tokens], 6 PSUM groups per chunk.
  - RoPE on eviction, all on DVE reading PSUM directly (4 ops/group):
      tmp[0:64]  = psum[64:128] * sinS[0:64]   (sinS first half = -sin)
      tmp[64:]   = psum[0:64]   * sinS[64:]    (second half = +sin)
      tmp2       = psum * cos
      dst(bf16)  = tmp + tmp2
  - scores transposed (sT[s,t] = kT_tile.T @ qT), exp fused into PSUM
    eviction on ACT (bf16 out); causal diagonal blocks are column-trimmed
    (ap = 512-128j) and masked by a 0/1 wedge multiply on DVE.
  - softmax denominator: lbc[m,t] += ones[128,128].T @ p  (all-ones
    stationary broadcasts the column sum to all 128 partitions, so the
    reciprocal needs no separate broadcast matmul).
  - normalize fused into av eviction: outT[:,h,:] = av_psum * recip(lbc).
  - o-projection contracts (h,H) against natural wo; evictions alternate
    ACT/DVE; output stays f32.
"""

import os
import sys

sys.path.insert(0, "/opt/trn_rl_repo")

import numpy as np

B = 2
T = 2048
TOK = B * T
D = 4096
NQ = 32
NKV = 8
H = 128
HH = H // 2
THETA = 10000.0
NCORES = 8
NHC = NQ // NCORES          # q heads per core (4)
KPC = D // H                # contraction chunks of 128 over D (32)
TCH = 512                   # token chunk
NTCH = T // TCH             # 4 token chunks per batch
NSUB = TCH // H             # 4 128-wide subtiles per chunk
C_SM = 1.0 / np.sqrt(H)     # softmax scale


def _build_bass():
    import concourse.bacc as bacc
    import concourse.mybir as mybir
    import concourse.tile as tile
    from concourse.masks import make_identity
    from contextlib import ExitStack

    f32 = mybir.dt.float32
    bf16 = mybir.dt.bfloat16
    Exp = mybir.ActivationFunctionType.Exp
    Copy = mybir.ActivationFunctionType.Copy

    nc = bacc.Bacc("TRN2", target_bir_lowering=False, debug=False,
                   num_devices=NCORES)

    xT = nc.declare_dram_parameter("xT", [D, TOK], bf16, isOutput=False)
    wq = nc.declare_dram_parameter("wq", [NHC, D, H], bf16, isOutput=False)
    wk = nc.declare_dram_parameter("wk", [D, H], bf16, isOutput=False)
    wv = nc.declare_dram_parameter("wv", [D, H], bf16, isOutput=False)
    wo = nc.declare_dram_parameter("wo", [NHC, H, D], bf16, isOutput=False)
    # rope tables duplicated across both partition halves; sinS's first
    # half is negated so rope reduces to dst = psum*cos + swap(psum)*sinS.
    cosT = nc.declare_dram_parameter("cosT", [H, TOK], f32, isOutput=False)
    sinST = nc.declare_dram_parameter("sinST", [H, TOK], f32, isOutput=False)
    o_part = nc.declare_dram_parameter("o_part", [TOK, D], f32, isOutput=True)

    with tile.TileContext(nc) as tc:
        with ExitStack() as top:
            consts = top.enter_context(tc.tile_pool(name="consts", bufs=1))
            identity = consts.tile([H, H], bf16)
            make_identity(nc, identity)
            ones_sq = consts.tile([H, H], bf16, tag="ones")
            nc.vector.memset(ones_sq, 1.0)
            # 0/1 causal wedge masks: mask[j][s, t] = 1 iff (t - s - 128j) >= 0
            masks = []
            for j in range(NSUB):
                m = consts.tile([H, TCH], bf16, tag=f"mask{j}",
                                name=f"mask{j}")
                nc.vector.memset(m, 1.0)
                nc.gpsimd.affine_select(
                    out=m, in_=m,
                    compare_op=mybir.AluOpType.is_ge,
                    fill=0.0,
                    base=-H * j,
                    pattern=[[1, TCH]],
                    channel_multiplier=-1,
                )
                masks.append(m)

            # ---- persistent weights / tables ----
            wpool = top.enter_context(tc.tile_pool(name="wpool", bufs=1))
            wq_src = wq.rearrange("h (c p) m -> p h c m", p=H)
            wqs = []
            for i in range(NHC):
                wq_h = wpool.tile([H, KPC, H], bf16, tag=f"wq{i}",
                                  name=f"wq{i}")
                for c8 in range(4):
                    sl = slice(c8 * 8, (c8 + 1) * 8)
                    nc.sync.dma_start(out=wq_h[:, sl, :],
                                      in_=wq_src[:, i, sl, :])
                wqs.append(wq_h)
            wk_sb = wpool.tile([H, KPC, H], bf16, tag="wk")
            wk_src = wk.rearrange("(c p) m -> p c m", p=H)
            wv_sb = wpool.tile([H, KPC, H], bf16, tag="wv")
            wv_src = wv.rearrange("(c p) m -> p c m", p=H)
            for c16 in range(2):
                sl = slice(c16 * 16, (c16 + 1) * 16)
                nc.sync.dma_start(out=wk_sb[:, sl, :], in_=wk_src[:, sl, :])
                nc.sync.dma_start(out=wv_sb[:, sl, :], in_=wv_src[:, sl, :])
            cos_sb = wpool.tile([H, TOK], f32, tag="cos")
            sin_sb = wpool.tile([H, TOK], f32, tag="sin")
            for c4 in range(4):
                sl = slice(c4 * 1024, (c4 + 1) * 1024)
                nc.sync.dma_start(out=cos_sb[:, sl], in_=cosT[:, sl])
                nc.sync.dma_start(out=sin_sb[:, sl], in_=sinST[:, sl])
            wo_sb = wpool.tile([H, NHC, D], bf16, tag="wo")
            wo_src = wo.rearrange("h p d -> p h d")
            for dc8 in range(8):
                sl = slice(dc8 * TCH, (dc8 + 1) * TCH)
                nc.sync.dma_start(out=wo_sb[:, :, sl], in_=wo_src[:, :, sl])

            # ---- persistent activations ----
            act = top.enter_context(tc.tile_pool(name="act", bufs=1))
            qTs = [act.tile([H, NHC, TCH], bf16, tag=f"qT{i}",
                            name=f"qT{i}") for i in range(NTCH)]
            kTs = [act.tile([H, TCH], bf16, tag=f"kT{i}",
                            name=f"kT{i}") for i in range(NTCH)]
            vs = [act.tile([H, NSUB, H], bf16, tag=f"v{i}",
                           name=f"v{i}") for i in range(NTCH)]

            xpool = top.enter_context(tc.tile_pool(name="xpool", bufs=12))
            rtmp = top.enter_context(tc.tile_pool(name="rtmp", bufs=2))
            vstg = top.enter_context(tc.tile_pool(name="vstg", bufs=2))
            ppool = top.enter_context(tc.tile_pool(name="ppool", bufs=4))
            pdiag = top.enter_context(tc.tile_pool(name="pdiag", bufs=2))
            rlpool = top.enter_context(tc.tile_pool(name="rlpool", bufs=2))
            otpool = top.enter_context(tc.tile_pool(name="otpool", bufs=2))
            opool = top.enter_context(tc.tile_pool(name="opool", bufs=4))

            def rope(psum, dst, cs, sn):
                tmp = rtmp.tile([H, TCH], f32, tag="rt")
                tmp2 = rtmp.tile([H, TCH], f32, tag="rt2")
                nc.vector.tensor_mul(tmp[0:HH, :], psum[HH:H, :], sn[0:HH, :])
                nc.vector.tensor_mul(tmp[HH:H, :], psum[0:HH, :], sn[HH:H, :])
                nc.vector.tensor_mul(tmp2, psum, cs)
                nc.vector.tensor_add(dst, tmp, tmp2)

            def emit_oproj(pend, po):
                pb, pc, outT = pend
                for u in range(NSUB):
                    trow = pb * T + pc * TCH + u * H
                    for dc in range(D // TCH):
                        ops = po.tile([H, TCH], f32, tag="o")
                        for h in range(NHC):
                            nc.tensor.matmul(
                                ops,
                                outT[:, h, u * H:(u + 1) * H],
                                wo_sb[:, h, dc * TCH:(dc + 1) * TCH],
                                start=(h == 0), stop=(h == NHC - 1),
                                skip_group_check=True)
                        o_sb = opool.tile([H, TCH], f32, tag="osb")
                        nc.scalar.activation(o_sb, ops, Copy)
                        nc.sync.dma_start(
                            out=o_part[trow:trow + H,
                                       dc * TCH:(dc + 1) * TCH],
                            in_=o_sb)

            pending = None
            for b in range(B):
                tb = b * T
                for c in range(NTCH):
                    t0 = tb + c * TCH
                    cs = cos_sb[:, t0:t0 + TCH]
                    sn = sin_sb[:, t0:t0 + TCH]
                    with ExitStack() as s1:
                        pj = s1.enter_context(
                            tc.tile_pool(name="pj", bufs=1, space="PSUM"))
                        po = s1.enter_context(
                            tc.tile_pool(name="po", bufs=2, space="PSUM"))
                        g = [pj.tile([H, TCH], f32, tag=f"g{i}",
                                     name=f"g{i}") for i in range(6)]
                        for k in range(KPC):
                            x_t = xpool.tile([H, TCH], bf16, tag="x")
                            nc.sync.dma_start(
                                out=x_t,
                                in_=xT[k * H:(k + 1) * H, t0:t0 + TCH])
                            lhs = [wqs[0][:, k, :], wqs[1][:, k, :],
                                   wqs[2][:, k, :], wqs[3][:, k, :],
                                   wk_sb[:, k, :], wv_sb[:, k, :]]
                            for i in range(6):
                                nc.tensor.matmul(
                                    g[i], lhs[i], x_t,
                                    start=(k == 0), stop=(k == KPC - 1),
                                    skip_group_check=True)
                        # evictions first in ACT/DVE queue order: v staging
                        # (ACT) and rope (DVE) so the attention inputs are
                        # ready early; the o-projection's PE work then
                        # covers them, with its o_sb evictions (ACT) queued
                        # behind and paced by its own matmuls.
                        vt_stage = vstg.tile([H, TCH], bf16, tag="vstage")
                        nc.scalar.activation(vt_stage, g[5], Copy)
                        rope(g[0], qTs[c][:, 0, :], cs, sn)
                        rope(g[4], kTs[c], cs, sn)
                        rope(g[1], qTs[c][:, 1, :], cs, sn)
                        rope(g[2], qTs[c][:, 2, :], cs, sn)
                        rope(g[3], qTs[c][:, 3, :], cs, sn)
                        if pending is not None:
                            emit_oproj(pending, po)
                            pending = None
                    with ExitStack() as s2:
                        pt = s2.enter_context(
                            tc.tile_pool(name="pt", bufs=2, space="PSUM"))
                        ps_s = s2.enter_context(
                            tc.tile_pool(name="ps_s", bufs=2, space="PSUM"))
                        ps_av = s2.enter_context(
                            tc.tile_pool(name="ps_av", bufs=2, space="PSUM"))
                        ps_l = s2.enter_context(
                            tc.tile_pool(name="ps_l", bufs=2, space="PSUM"))
                        for j in range(NSUB):
                            tp = pt.tile([H, H], bf16, tag="vtp")
                            nc.tensor.transpose(
                                tp, vt_stage[:, j * H:(j + 1) * H], identity)
                            nc.scalar.activation(vs[c][:, j, :], tp, Copy)
                        n_st = (c + 1) * NSUB
                        outT_sb = otpool.tile([H, NHC, TCH], bf16, tag="outT")
                        for h in range(NHC):
                            av_ps = ps_av.tile([H, TCH], f32, tag="av")
                            l_ps = ps_l.tile([H, TCH], f32, tag="lbc")

                            def scores_block(st):
                                j = st - c * NSUB
                                off = H * j if j > 0 else 0
                                kt = kTs[st // NSUB][
                                    :, (st % NSUB) * H:(st % NSUB + 1) * H]
                                sps = ps_s.tile([H, TCH], f32, tag="s")
                                nc.tensor.matmul(
                                    sps[:, off:], kt, qTs[c][:, h, off:],
                                    start=True, stop=True)
                                pT2 = ppool.tile([H, TCH], bf16, tag="p2")
                                if j >= 0:
                                    pT = pdiag.tile([H, TCH], bf16, tag="pd")
                                    nc.scalar.activation(
                                        pT[:, off:], sps[:, off:], Exp,
                                        scale=C_SM)
                                    nc.vector.tensor_mul(
                                        pT2[:, off:], pT[:, off:],
                                        masks[j][:, off:])
                                else:
                                    nc.scalar.activation(
                                        pT2, sps, Exp, scale=C_SM)
                                return pT2

                            def av_block(st, pT2):
                                j = st - c * NSUB
                                off = H * j if j > 0 else 0
                                nc.tensor.matmul(
                                    av_ps[:, off:],
                                    vs[st // NSUB][:, st % NSUB, :],
                                    pT2[:, off:],
                                    start=(st == 0), stop=(st == n_st - 1),
                                    skip_group_check=True)
                                nc.tensor.matmul(
                                    l_ps[:, off:], ones_sq, pT2[:, off:],
                                    start=(st == 0), stop=(st == n_st - 1),
                                    skip_group_check=True)

                            prev = scores_block(0)
                            for st in range(1, n_st):
                                cur = scores_block(st)
                                av_block(st - 1, prev)
                                prev = cur
                            av_block(n_st - 1, prev)
                            rl = rlpool.tile([H, TCH], f32, tag="rl")
                            nc.vector.reciprocal(rl, l_ps)
                            nc.vector.tensor_mul(outT_sb[:, h, :], av_ps, rl)
                        pending = (b, c, outT_sb)
            with ExitStack() as s3:
                po = s3.enter_context(
                    tc.tile_pool(name="po3", bufs=2, space="PSUM"))
                emit_oproj(pending, po)

    nc.compile()
    return nc


_NC_CACHE = None


def kernel(x, wq, wk, wv, wo, positions):
    global _NC_CACHE
    import ml_dtypes
    from concourse.bass_utils import run_bass_kernel_spmd

    bf16 = ml_dtypes.bfloat16
    x = np.asarray(x, dtype=np.float32)
    wq = np.asarray(wq, dtype=np.float32)
    wk = np.asarray(wk, dtype=np.float32)
    wv = np.asarray(wv, dtype=np.float32)
    wo = np.asarray(wo, dtype=np.float32)
    positions = np.asarray(positions)

    xT = np.ascontiguousarray(x.reshape(TOK, D).T).astype(bf16)
    # rope tables, [H, B*T]: duplicated across partition halves; sin's
    # first half negated (see kernel docstring).
    fraction = 2.0 * np.arange(HH, dtype=np.float32) / H
    timescale = (THETA ** fraction).astype(np.float32)
    pos = positions.reshape(TOK).astype(np.float32)
    sinusoid = pos[None, :] / timescale[:, None]
    cos_h = np.cos(sinusoid).astype(np.float32)
    sin_h = np.sin(sinusoid).astype(np.float32)
    cosT = np.ascontiguousarray(np.concatenate([cos_h, cos_h], axis=0))
    sinST = np.ascontiguousarray(np.concatenate([-sin_h, sin_h], axis=0))

    if _NC_CACHE is None:
        _NC_CACHE = _build_bass()
    nc = _NC_CACHE

    in_maps = []
    for c in range(NCORES):
        in_maps.append({
            "xT": xT,
            "wq": np.ascontiguousarray(wq[c * NHC:(c + 1) * NHC]).astype(bf16),
            "wk": np.ascontiguousarray(wk[c]).astype(bf16),
            "wv": np.ascontiguousarray(wv[c]).astype(bf16),
            "wo": np.ascontiguousarray(wo[c * NHC:(c + 1) * NHC]).astype(bf16),
            "cosT": cosT,
            "sinST": sinST,
        })

    trace = os.environ.get("BASS_KERNEL_TRACE", "0") == "1"
    res = run_bass_kernel_spmd(nc, in_maps, list(range(NCORES)), trace=trace)
    global LAST_RESULTS
    LAST_RESULTS = res
    out = np.zeros((TOK, D), dtype=np.float32)
    for c in range(NCORES):
        out += res.results[c]["o_part"]
    return out.reshape(B, T, D)


LAST_RESULTS = None


# revision 8
# speedup vs baseline: 1.1179x; 1.1179x over previous
"""GQA causal-attention prefill kernel for Trainium2, tensor-parallel over 8 NeuronCores.

v2: all-bf16 datapath, fused single-stream schedule.

Sharding: head-parallel. Core c gets q heads [4c, 4c+4), kv head c, and the
matching wo slice; each core computes a full-shape partial output
o_part = attn(heads of c) @ wo_c and the host sums the 8 partials.

Per-core schedule (one PE stream, no phase barriers):
  for b, c:   PROJ(b,c) k-sweep -> OPROJ(prev chunk) -> ATTN(b,c)
The o-projection of the previous chunk fills the PE while the current
chunk's rope evictions run on DVE, so the PE never waits on the rope tail.

Layouts:
  - xT [D, B*T] bf16; projections contract D on the partition dim:
    qT/kT/vT[h] = w[h].T @ xT -> [H=128, tokens], 6 PSUM groups per chunk.
  - RoPE on eviction, all on DVE reading PSUM directly (4 ops/group):
      tmp[0:64]  = psum[64:128] * sinS[0:64]   (sinS first half = -sin)
      tmp[64:]   = psum[0:64]   * sinS[64:]    (second half = +sin)
      tmp2       = psum * cos
      dst(bf16)  = tmp + tmp2
  - scores transposed (sT[s,t] = kT_tile.T @ qT), exp fused into PSUM
    eviction on ACT (bf16 out); causal diagonal blocks are column-trimmed
    (ap = 512-128j) and masked by a 0/1 wedge multiply on DVE.
  - softmax denominator: lbc[m,t] += ones[128,128].T @ p  (all-ones
    stationary broadcasts the column sum to all 128 partitions, so the
    reciprocal needs no separate broadcast matmul).
  - normalize fused into av eviction: outT[:,h,:] = av_psum * recip(lbc).
  - o-projection contracts (h,H) against natural wo; evictions alternate
    ACT/DVE; output stays f32.
"""

import os
import sys

sys.path.insert(0, "/opt/trn_rl_repo")

import numpy as np

B = 2
T = 2048
TOK = B * T
D = 4096
NQ = 32
NKV = 8
H = 128
HH = H // 2
THETA = 10000.0
NCORES = 8
NHC = NQ // NCORES          # q heads per core (4)
KPC = D // H                # contraction chunks of 128 over D (32)
TCH = 512                   # token chunk
NTCH = T // TCH             # 4 token chunks per batch
NSUB = TCH // H             # 4 128-wide subtiles per chunk
C_SM = 1.0 / np.sqrt(H)     # softmax scale


def _build_bass():
    import concourse.bacc as bacc
    import concourse.mybir as mybir
    import concourse.tile as tile
    from concourse.masks import make_identity
    from contextlib import ExitStack

    f32 = mybir.dt.float32
    bf16 = mybir.dt.bfloat16
    Exp = mybir.ActivationFunctionType.Exp
    Ln = mybir.ActivationFunctionType.Ln
    Copy = mybir.ActivationFunctionType.Copy

    nc = bacc.Bacc("TRN2", target_bir_lowering=False, debug=False,
                   num_devices=NCORES)

    xT = nc.declare_dram_parameter("xT", [D, TOK], bf16, isOutput=False)
    wq = nc.declare_dram_parameter("wq", [NHC, D, H], bf16, isOutput=False)
    wk = nc.declare_dram_parameter("wk", [D, H], bf16, isOutput=False)
    wv = nc.declare_dram_parameter("wv", [D, H], bf16, isOutput=False)
    wo = nc.declare_dram_parameter("wo", [NHC, H, D], bf16, isOutput=False)
    # rope tables duplicated across both partition halves; sinS's first
    # half is negated so rope reduces to dst = psum*cos + swap(psum)*sinS.
    cosT = nc.declare_dram_parameter("cosT", [H, TOK], f32, isOutput=False)
    sinST = nc.declare_dram_parameter("sinST", [H, TOK], f32, isOutput=False)
    o_part = nc.declare_dram_parameter("o_part", [TOK, D], bf16,
                                      isOutput=True)

    with tile.TileContext(nc) as tc:
        with ExitStack() as top:
            consts = top.enter_context(tc.tile_pool(name="consts", bufs=1))
            identity = consts.tile([H, H], bf16)
            make_identity(nc, identity)
            ones_sq = consts.tile([H, H], bf16, tag="ones")
            nc.vector.memset(ones_sq, 1.0)
            # 0/1 causal wedge masks: mask[j][s, t] = 1 iff (t - s - 128j) >= 0
            masks = []
            for j in range(NSUB):
                m = consts.tile([H, TCH], bf16, tag=f"mask{j}",
                                name=f"mask{j}")
                nc.vector.memset(m, 1.0)
                nc.gpsimd.affine_select(
                    out=m, in_=m,
                    compare_op=mybir.AluOpType.is_ge,
                    fill=0.0,
                    base=-H * j,
                    pattern=[[1, TCH]],
                    channel_multiplier=-1,
                )
                masks.append(m)

            # ---- persistent weights / tables ----
            wpool = top.enter_context(tc.tile_pool(name="wpool", bufs=1))
            wq_src = wq.rearrange("h (c p) m -> p h c m", p=H)
            wqs = []
            for i in range(NHC):
                wq_h = wpool.tile([H, KPC, H], bf16, tag=f"wq{i}",
                                  name=f"wq{i}")
                for c8 in range(4):
                    sl = slice(c8 * 8, (c8 + 1) * 8)
                    nc.sync.dma_start(out=wq_h[:, sl, :],
                                      in_=wq_src[:, i, sl, :])
                wqs.append(wq_h)
            wk_sb = wpool.tile([H, KPC, H], bf16, tag="wk")
            wk_src = wk.rearrange("(c p) m -> p c m", p=H)
            wv_sb = wpool.tile([H, KPC, H], bf16, tag="wv")
            wv_src = wv.rearrange("(c p) m -> p c m", p=H)
            for c16 in range(2):
                sl = slice(c16 * 16, (c16 + 1) * 16)
                nc.sync.dma_start(out=wk_sb[:, sl, :], in_=wk_src[:, sl, :])
                nc.sync.dma_start(out=wv_sb[:, sl, :], in_=wv_src[:, sl, :])
            cos_sb = wpool.tile([H, TOK], f32, tag="cos")
            sin_sb = wpool.tile([H, TOK], f32, tag="sin")
            for c4 in range(4):
                sl = slice(c4 * 1024, (c4 + 1) * 1024)
                nc.sync.dma_start(out=cos_sb[:, sl], in_=cosT[:, sl])
                nc.sync.dma_start(out=sin_sb[:, sl], in_=sinST[:, sl])
            wo_sb = wpool.tile([H, NHC, D], bf16, tag="wo")
            wo_src = wo.rearrange("h p d -> p h d")
            for dc8 in range(8):
                sl = slice(dc8 * TCH, (dc8 + 1) * TCH)
                nc.sync.dma_start(out=wo_sb[:, :, sl], in_=wo_src[:, :, sl])

            # ---- persistent activations ----
            act = top.enter_context(tc.tile_pool(name="act", bufs=1))
            qTs = [act.tile([H, NHC, TCH], bf16, tag=f"qT{i}",
                            name=f"qT{i}") for i in range(NTCH)]
            kTs = [act.tile([H, TCH], bf16, tag=f"kT{i}",
                            name=f"kT{i}") for i in range(NTCH)]
            vs = [act.tile([H, NSUB, H], bf16, tag=f"v{i}",
                           name=f"v{i}") for i in range(NTCH)]

            xpool = top.enter_context(tc.tile_pool(name="xpool", bufs=16))
            rtmp = top.enter_context(tc.tile_pool(name="rtmp", bufs=2))
            vstg = top.enter_context(tc.tile_pool(name="vstg", bufs=2))
            ppool = top.enter_context(tc.tile_pool(name="ppool", bufs=4))
            pdiag = top.enter_context(tc.tile_pool(name="pdiag", bufs=2))
            rlpool = top.enter_context(tc.tile_pool(name="rlpool", bufs=2))
            otpool = top.enter_context(tc.tile_pool(name="otpool", bufs=2))
            opool = top.enter_context(tc.tile_pool(name="opool", bufs=4))

            def rope(psum, dst, cs, sn):
                tmp = rtmp.tile([H, TCH], f32, tag="rt")
                tmp2 = rtmp.tile([H, TCH], f32, tag="rt2")
                nc.vector.tensor_mul(tmp[0:HH, :], psum[HH:H, :], sn[0:HH, :])
                nc.vector.tensor_mul(tmp[HH:H, :], psum[0:HH, :], sn[HH:H, :])
                nc.vector.tensor_mul(tmp2, psum, cs)
                nc.vector.tensor_add(dst, tmp, tmp2)

            def emit_oproj(pend, po):
                pb, pc, outT = pend
                for u in range(NSUB):
                    trow = pb * T + pc * TCH + u * H
                    for dc in range(D // TCH):
                        ops = po.tile([H, TCH], f32, tag="o")
                        for h in range(NHC):
                            nc.tensor.matmul(
                                ops,
                                outT[:, h, u * H:(u + 1) * H],
                                wo_sb[:, h, dc * TCH:(dc + 1) * TCH],
                                start=(h == 0), stop=(h == NHC - 1),
                                skip_group_check=True)
                        o_sb = opool.tile([H, TCH], bf16, tag="osb")
                        nc.scalar.activation(o_sb, ops, Copy)
                        nc.sync.dma_start(
                            out=o_part[trow:trow + H,
                                       dc * TCH:(dc + 1) * TCH],
                            in_=o_sb)

            pending = None
            for b in range(B):
                tb = b * T
                for c in range(NTCH):
                    t0 = tb + c * TCH
                    cs = cos_sb[:, t0:t0 + TCH]
                    sn = sin_sb[:, t0:t0 + TCH]
                    with ExitStack() as s1:
                        pj = s1.enter_context(
                            tc.tile_pool(name="pj", bufs=1, space="PSUM"))
                        po = s1.enter_context(
                            tc.tile_pool(name="po", bufs=2, space="PSUM"))
                        g = [pj.tile([H, TCH], f32, tag=f"g{i}",
                                     name=f"g{i}") for i in range(6)]
                        for k in range(KPC):
                            x_t = xpool.tile([H, TCH], bf16, tag="x")
                            nc.sync.dma_start(
                                out=x_t,
                                in_=xT[k * H:(k + 1) * H, t0:t0 + TCH])
                            lhs = [wqs[0][:, k, :], wqs[1][:, k, :],
                                   wqs[2][:, k, :], wqs[3][:, k, :],
                                   wk_sb[:, k, :], wv_sb[:, k, :]]
                            for i in range(6):
                                nc.tensor.matmul(
                                    g[i], lhs[i], x_t,
                                    start=(k == 0), stop=(k == KPC - 1),
                                    skip_group_check=True)
                        # evictions first in ACT/DVE queue order: v staging
                        # (ACT) and rope (DVE) so the attention inputs are
                        # ready early; the o-projection's PE work then
                        # covers them, with its o_sb evictions (ACT) queued
                        # behind and paced by its own matmuls.
                        vt_stage = vstg.tile([H, TCH], bf16, tag="vstage")
                        nc.scalar.activation(vt_stage, g[5], Copy)
                        rope(g[0], qTs[c][:, 0, :], cs, sn)
                        rope(g[4], kTs[c], cs, sn)
                        rope(g[1], qTs[c][:, 1, :], cs, sn)
                        rope(g[2], qTs[c][:, 2, :], cs, sn)
                        rope(g[3], qTs[c][:, 3, :], cs, sn)
                        if pending is not None:
                            emit_oproj(pending, po)
                            pending = None
                    with ExitStack() as s2:
                        pt = s2.enter_context(
                            tc.tile_pool(name="pt", bufs=2, space="PSUM"))
                        ps_s = s2.enter_context(
                            tc.tile_pool(name="ps_s", bufs=2, space="PSUM"))
                        ps_av = s2.enter_context(
                            tc.tile_pool(name="ps_av", bufs=2, space="PSUM"))
                        ps_l = s2.enter_context(
                            tc.tile_pool(name="ps_l", bufs=2, space="PSUM"))
                        for j in range(NSUB):
                            tp = pt.tile([H, H], bf16, tag="vtp")
                            nc.tensor.transpose(
                                tp, vt_stage[:, j * H:(j + 1) * H], identity)
                            nc.scalar.activation(vs[c][:, j, :], tp, Copy)
                        n_st = (c + 1) * NSUB
                        outT_sb = otpool.tile([H, NHC, TCH], bf16, tag="outT")
                        for h in range(NHC):
                            av_ps = ps_av.tile([H, TCH], f32, tag="av")
                            l_ps = ps_l.tile([H, TCH], f32, tag="lbc")

                            def scores_block(st):
                                j = st - c * NSUB
                                off = H * j if j > 0 else 0
                                kt = kTs[st // NSUB][
                                    :, (st % NSUB) * H:(st % NSUB + 1) * H]
                                sps = ps_s.tile([H, TCH], f32, tag="s")
                                nc.tensor.matmul(
                                    sps[:, off:], kt, qTs[c][:, h, off:],
                                    start=True, stop=True)
                                pT2 = ppool.tile([H, TCH], bf16, tag="p2")
                                if j >= 0:
                                    pT = pdiag.tile([H, TCH], bf16, tag="pd")
                                    nc.scalar.activation(
                                        pT[:, off:], sps[:, off:], Exp,
                                        scale=C_SM)
                                    nc.vector.tensor_mul(
                                        pT2[:, off:], pT[:, off:],
                                        masks[j][:, off:])
                                else:
                                    nc.scalar.activation(
                                        pT2, sps, Exp, scale=C_SM)
                                return pT2

                            def av_block(st, pT2):
                                j = st - c * NSUB
                                off = H * j if j > 0 else 0
                                nc.tensor.matmul(
                                    av_ps[:, off:],
                                    vs[st // NSUB][:, st % NSUB, :],
                                    pT2[:, off:],
                                    start=(st == 0), stop=(st == n_st - 1),
                                    skip_group_check=True)
                                nc.tensor.matmul(
                                    l_ps[:, off:], ones_sq, pT2[:, off:],
                                    start=(st == 0), stop=(st == n_st - 1),
                                    skip_group_check=True)

                            prev = scores_block(0)
                            for st in range(1, n_st):
                                cur = scores_block(st)
                                av_block(st - 1, prev)
                                prev = cur
                            av_block(n_st - 1, prev)
                            # 1/l as exp(-ln(l)) on ACT: DVE reciprocal is
                            # ~8 cycles/elem (4us per [128,512], serializing
                            # the per-head chain); two ACT passes are 1.3us
                            # and off the DVE critical path.
                            lg = rlpool.tile([H, TCH], f32, tag="lg")
                            nc.scalar.activation(lg, l_ps, Ln)
                            rl = rlpool.tile([H, TCH], f32, tag="rl")
                            nc.scalar.activation(rl, lg, Exp, scale=-1.0)
                            nc.vector.tensor_mul(outT_sb[:, h, :], av_ps, rl)
                        pending = (b, c, outT_sb)
            with ExitStack() as s3:
                po = s3.enter_context(
                    tc.tile_pool(name="po3", bufs=2, space="PSUM"))
                emit_oproj(pending, po)

    nc.compile()
    return nc


_NC_CACHE = None


def kernel(x, wq, wk, wv, wo, positions):
    global _NC_CACHE
    import ml_dtypes
    from concourse.bass_utils import run_bass_kernel_spmd

    bf16 = ml_dtypes.bfloat16
    x = np.asarray(x, dtype=np.float32)
    wq = np.asarray(wq, dtype=np.float32)
    wk = np.asarray(wk, dtype=np.float32)
    wv = np.asarray(wv, dtype=np.float32)
    wo = np.asarray(wo, dtype=np.float32)
    positions = np.asarray(positions)

    xT = np.ascontiguousarray(x.reshape(TOK, D).T).astype(bf16)
    # rope tables, [H, B*T]: duplicated across partition halves; sin's
    # first half negated (see kernel docstring).
    fraction = 2.0 * np.arange(HH, dtype=np.float32) / H
    timescale = (THETA ** fraction).astype(np.float32)
    pos = positions.reshape(TOK).astype(np.float32)
    sinusoid = pos[None, :] / timescale[:, None]
    cos_h = np.cos(sinusoid).astype(np.float32)
    sin_h = np.sin(sinusoid).astype(np.float32)
    cosT = np.ascontiguousarray(np.concatenate([cos_h, cos_h], axis=0))
    sinST = np.ascontiguousarray(np.concatenate([-sin_h, sin_h], axis=0))

    if _NC_CACHE is None:
        _NC_CACHE = _build_bass()
    nc = _NC_CACHE

    in_maps = []
    for c in range(NCORES):
        in_maps.append({
            "xT": xT,
            "wq": np.ascontiguousarray(wq[c * NHC:(c + 1) * NHC]).astype(bf16),
            "wk": np.ascontiguousarray(wk[c]).astype(bf16),
            "wv": np.ascontiguousarray(wv[c]).astype(bf16),
            "wo": np.ascontiguousarray(wo[c * NHC:(c + 1) * NHC]).astype(bf16),
            "cosT": cosT,
            "sinST": sinST,
        })

    trace = os.environ.get("BASS_KERNEL_TRACE", "0") == "1"
    res = run_bass_kernel_spmd(nc, in_maps, list(range(NCORES)), trace=trace)
    global LAST_RESULTS
    LAST_RESULTS = res
    out = np.zeros((TOK, D), dtype=np.float32)
    for c in range(NCORES):
        out += res.results[c]["o_part"].astype(np.float32)
    return out.reshape(B, T, D)


LAST_RESULTS = None


# revision 12
# speedup vs baseline: 1.2236x; 1.0946x over previous
"""GQA causal-attention prefill kernel for Trainium2, tensor-parallel over 8 NeuronCores.

v2: all-bf16 datapath, fused single-stream schedule.

Sharding: head-parallel. Core c gets q heads [4c, 4c+4), kv head c, and the
matching wo slice; each core computes a full-shape partial output
o_part = attn(heads of c) @ wo_c and the host sums the 8 partials.

Per-core schedule (one PE stream, no phase barriers):
  for b, c:   PROJ(b,c) k-sweep -> OPROJ(prev chunk) -> ATTN(b,c)
The o-projection of the previous chunk fills the PE while the current
chunk's rope evictions run on DVE, so the PE never waits on the rope tail.

Layouts:
  - xT [D, B*T] bf16; projections contract D on the partition dim:
    qT/kT/vT[h] = w[h].T @ xT -> [H=128, tokens], 6 PSUM groups per chunk.
  - RoPE on eviction, all on DVE reading PSUM directly (4 ops/group):
      tmp[0:64]  = psum[64:128] * sinS[0:64]   (sinS first half = -sin)
      tmp[64:]   = psum[0:64]   * sinS[64:]    (second half = +sin)
      tmp2       = psum * cos
      dst(bf16)  = tmp + tmp2
  - scores transposed (sT[s,t] = kT_tile.T @ qT), exp fused into PSUM
    eviction on ACT (bf16 out); causal diagonal blocks are column-trimmed
    (ap = 512-128j) and masked by a 0/1 wedge multiply on DVE.
  - softmax denominator: lbc[m,t] += ones[128,128].T @ p  (all-ones
    stationary broadcasts the column sum to all 128 partitions, so the
    reciprocal needs no separate broadcast matmul).
  - normalize fused into av eviction: outT[:,h,:] = av_psum * recip(lbc).
  - o-projection contracts (h,H) against natural wo; evictions alternate
    ACT/DVE; output stays f32.
"""

import os
import sys

sys.path.insert(0, "/opt/trn_rl_repo")

import numpy as np

B = 2
T = 2048
TOK = B * T
D = 4096
NQ = 32
NKV = 8
H = 128
HH = H // 2
THETA = 10000.0
NCORES = 8
NHC = NQ // NCORES          # q heads per core (4)
KPC = D // H                # contraction chunks of 128 over D (32)
TCH = 512                   # token chunk
NTCH = T // TCH             # 4 token chunks per batch
NSUB = TCH // H             # 4 128-wide subtiles per chunk
C_SM = 1.0 / np.sqrt(H)     # softmax scale


def _build_bass():
    import concourse.bacc as bacc
    import concourse.mybir as mybir
    import concourse.tile as tile
    from concourse.masks import make_identity
    from contextlib import ExitStack

    f32 = mybir.dt.float32
    bf16 = mybir.dt.bfloat16
    Exp = mybir.ActivationFunctionType.Exp
    Ln = mybir.ActivationFunctionType.Ln
    Copy = mybir.ActivationFunctionType.Copy

    nc = bacc.Bacc("TRN2", target_bir_lowering=False, debug=False,
                   num_devices=NCORES)

    xT = nc.declare_dram_parameter("xT", [D, TOK], bf16, isOutput=False)
    wq = nc.declare_dram_parameter("wq", [NHC, D, H], bf16, isOutput=False)
    wk = nc.declare_dram_parameter("wk", [D, H], bf16, isOutput=False)
    wv = nc.declare_dram_parameter("wv", [D, H], bf16, isOutput=False)
    wo = nc.declare_dram_parameter("wo", [NHC, H, D], bf16, isOutput=False)
    # rope tables duplicated across both partition halves; sinS's first
    # half is negated so rope reduces to dst = psum*cos + swap(psum)*sinS.
    cosT = nc.declare_dram_parameter("cosT", [H, TOK], f32, isOutput=False)
    sinST = nc.declare_dram_parameter("sinST", [H, TOK], f32, isOutput=False)
    o_part = nc.declare_dram_parameter("o_part", [TOK, D], bf16,
                                      isOutput=True)

    with tile.TileContext(nc) as tc:
        with ExitStack() as top:
            consts = top.enter_context(tc.tile_pool(name="consts", bufs=1))
            identity = consts.tile([H, H], bf16)
            make_identity(nc, identity)
            ones_sq = consts.tile([H, H], bf16, tag="ones")
            nc.vector.memset(ones_sq, 1.0)
            # 0/1 causal wedge masks: mask[j][s, t] = 1 iff (t - s - 128j) >= 0
            masks = []
            for j in range(NSUB):
                m = consts.tile([H, TCH], bf16, tag=f"mask{j}",
                                name=f"mask{j}")
                nc.vector.memset(m, 1.0)
                nc.gpsimd.affine_select(
                    out=m, in_=m,
                    compare_op=mybir.AluOpType.is_ge,
                    fill=0.0,
                    base=-H * j,
                    pattern=[[1, TCH]],
                    channel_multiplier=-1,
                )
                masks.append(m)

            # ---- persistent weights / tables ----
            wpool = top.enter_context(tc.tile_pool(name="wpool", bufs=1))
            wq_src = wq.rearrange("h (c p) m -> p h c m", p=H)
            wqs = []
            for i in range(NHC):
                wq_h = wpool.tile([H, KPC, H], bf16, tag=f"wq{i}",
                                  name=f"wq{i}")
                for c8 in range(4):
                    sl = slice(c8 * 8, (c8 + 1) * 8)
                    nc.sync.dma_start(out=wq_h[:, sl, :],
                                      in_=wq_src[:, i, sl, :])
                wqs.append(wq_h)
            wk_sb = wpool.tile([H, KPC, H], bf16, tag="wk")
            wk_src = wk.rearrange("(c p) m -> p c m", p=H)
            wv_sb = wpool.tile([H, KPC, H], bf16, tag="wv")
            wv_src = wv.rearrange("(c p) m -> p c m", p=H)
            for c16 in range(2):
                sl = slice(c16 * 16, (c16 + 1) * 16)
                nc.sync.dma_start(out=wk_sb[:, sl, :], in_=wk_src[:, sl, :])
                nc.sync.dma_start(out=wv_sb[:, sl, :], in_=wv_src[:, sl, :])
            cos_sb = wpool.tile([H, TOK], f32, tag="cos")
            sin_sb = wpool.tile([H, TOK], f32, tag="sin")
            for c4 in range(4):
                sl = slice(c4 * 1024, (c4 + 1) * 1024)
                nc.sync.dma_start(out=cos_sb[:, sl], in_=cosT[:, sl])
                nc.sync.dma_start(out=sin_sb[:, sl], in_=sinST[:, sl])
            wo_sb = wpool.tile([H, NHC, D], bf16, tag="wo")
            wo_src = wo.rearrange("h p d -> p h d")
            for dc8 in range(8):
                sl = slice(dc8 * TCH, (dc8 + 1) * TCH)
                nc.sync.dma_start(out=wo_sb[:, :, sl], in_=wo_src[:, :, sl])

            # ---- persistent activations ----
            act = top.enter_context(tc.tile_pool(name="act", bufs=1))
            qTs = [act.tile([H, NHC, TCH], bf16, tag=f"qT{i}",
                            name=f"qT{i}") for i in range(NTCH)]
            kTs = [act.tile([H, TCH], bf16, tag=f"kT{i}",
                            name=f"kT{i}") for i in range(NTCH)]
            vs = [act.tile([H, NSUB, H], bf16, tag=f"v{i}",
                           name=f"v{i}") for i in range(NTCH)]

            xpool = top.enter_context(tc.tile_pool(name="xpool", bufs=16))
            rtmp = top.enter_context(tc.tile_pool(name="rtmp", bufs=2))
            vstg = top.enter_context(tc.tile_pool(name="vstg", bufs=2))
            ppool = top.enter_context(tc.tile_pool(name="ppool", bufs=4))
            pdiag = top.enter_context(tc.tile_pool(name="pdiag", bufs=2))
            avlpool = top.enter_context(tc.tile_pool(name="avlpool", bufs=1))
            otpool = top.enter_context(tc.tile_pool(name="otpool", bufs=2))
            opool = top.enter_context(tc.tile_pool(name="opool", bufs=3))

            def rope(psum, dst, cs, sn):
                tmp = rtmp.tile([H, TCH], f32, tag="rt")
                tmp2 = rtmp.tile([H, TCH], f32, tag="rt2")
                nc.vector.tensor_mul(tmp[0:HH, :], psum[HH:H, :], sn[0:HH, :])
                nc.vector.tensor_mul(tmp[HH:H, :], psum[0:HH, :], sn[HH:H, :])
                nc.vector.tensor_mul(tmp2, psum, cs)
                nc.vector.tensor_add(dst, tmp, tmp2)

            def emit_oproj(pend, po):
                pb, pc, outT = pend
                for u in range(NSUB):
                    trow = pb * T + pc * TCH + u * H
                    for dc in range(D // TCH):
                        ops = po.tile([H, TCH], f32, tag="o")
                        for h in range(NHC):
                            nc.tensor.matmul(
                                ops,
                                outT[:, h, u * H:(u + 1) * H],
                                wo_sb[:, h, dc * TCH:(dc + 1) * TCH],
                                start=(h == 0), stop=(h == NHC - 1),
                                skip_group_check=True)
                        o_sb = opool.tile([H, TCH], bf16, tag="osb")
                        nc.scalar.activation(o_sb, ops, Copy)
                        nc.sync.dma_start(
                            out=o_part[trow:trow + H,
                                       dc * TCH:(dc + 1) * TCH],
                            in_=o_sb)

            pending = None
            for b in range(B):
                tb = b * T
                for c in range(NTCH):
                    t0 = tb + c * TCH
                    cs = cos_sb[:, t0:t0 + TCH]
                    sn = sin_sb[:, t0:t0 + TCH]
                    with ExitStack() as s1:
                        pj = s1.enter_context(
                            tc.tile_pool(name="pj", bufs=1, space="PSUM"))
                        po = s1.enter_context(
                            tc.tile_pool(name="po", bufs=2, space="PSUM"))
                        g = [pj.tile([H, TCH], f32, tag=f"g{i}",
                                     name=f"g{i}") for i in range(6)]
                        for k in range(KPC):
                            x_t = xpool.tile([H, TCH], bf16, tag="x")
                            nc.sync.dma_start(
                                out=x_t,
                                in_=xT[k * H:(k + 1) * H, t0:t0 + TCH])
                            lhs = [wqs[0][:, k, :], wqs[1][:, k, :],
                                   wqs[2][:, k, :], wqs[3][:, k, :],
                                   wk_sb[:, k, :], wv_sb[:, k, :]]
                            for i in range(6):
                                nc.tensor.matmul(
                                    g[i], lhs[i], x_t,
                                    start=(k == 0), stop=(k == KPC - 1),
                                    skip_group_check=True)
                        # evictions first in ACT/DVE queue order: v staging
                        # (ACT) and rope (DVE) so the attention inputs are
                        # ready early; the o-projection's PE work then
                        # covers them, with its o_sb evictions (ACT) queued
                        # behind and paced by its own matmuls.
                        vt_stage = vstg.tile([H, TCH], bf16, tag="vstage")
                        nc.scalar.activation(vt_stage, g[5], Copy)
                        rope(g[0], qTs[c][:, 0, :], cs, sn)
                        rope(g[4], kTs[c], cs, sn)
                        rope(g[1], qTs[c][:, 1, :], cs, sn)
                        rope(g[2], qTs[c][:, 2, :], cs, sn)
                        rope(g[3], qTs[c][:, 3, :], cs, sn)
                        if pending is not None:
                            emit_oproj(pending, po)
                            pending = None
                    with ExitStack() as s2:
                        pt = s2.enter_context(
                            tc.tile_pool(name="pt", bufs=2, space="PSUM"))
                        ps_s = s2.enter_context(
                            tc.tile_pool(name="ps_s", bufs=2, space="PSUM"))
                        ps_av = s2.enter_context(
                            tc.tile_pool(name="ps_av", bufs=2, space="PSUM"))
                        ps_l = s2.enter_context(
                            tc.tile_pool(name="ps_l", bufs=2, space="PSUM"))
                        for j in range(NSUB):
                            tp = pt.tile([H, H], bf16, tag="vtp")
                            nc.tensor.transpose(
                                tp, vt_stage[:, j * H:(j + 1) * H], identity)
                            nc.scalar.activation(vs[c][:, j, :], tp, Copy)
                        n_st = (c + 1) * NSUB
                        outT_sb = otpool.tile([H, NHC, TCH], bf16, tag="outT")
                        # per-head av/l leave PSUM via plain ACT copies (no
                        # activation-table switch); the Ln/Exp reciprocal is
                        # batched once per chunk so the Exp<->Ln table reload
                        # (1.3us each) happens 2x per chunk instead of 2x
                        # per head, off the per-head critical path.
                        av_all = avlpool.tile([H, NHC, TCH], bf16, tag="ava")
                        l_all = avlpool.tile([H, NHC, TCH], bf16, tag="la")
                        for h in range(NHC):
                            av_ps = ps_av.tile([H, TCH], f32, tag="av")
                            l_ps = ps_l.tile([H, TCH], f32, tag="lbc")

                            def scores_block(st):
                                j = st - c * NSUB
                                off = H * j if j > 0 else 0
                                kt = kTs[st // NSUB][
                                    :, (st % NSUB) * H:(st % NSUB + 1) * H]
                                sps = ps_s.tile([H, TCH], f32, tag="s")
                                nc.tensor.matmul(
                                    sps[:, off:], kt, qTs[c][:, h, off:],
                                    start=True, stop=True)
                                pT2 = ppool.tile([H, TCH], bf16, tag="p2")
                                if j >= 0:
                                    pT = pdiag.tile([H, TCH], bf16, tag="pd")
                                    nc.scalar.activation(
                                        pT[:, off:], sps[:, off:], Exp,
                                        scale=C_SM)
                                    nc.vector.tensor_mul(
                                        pT2[:, off:], pT[:, off:],
                                        masks[j][:, off:])
                                else:
                                    nc.scalar.activation(
                                        pT2, sps, Exp, scale=C_SM)
                                return pT2

                            def av_block(st, pT2):
                                j = st - c * NSUB
                                off = H * j if j > 0 else 0
                                nc.tensor.matmul(
                                    av_ps[:, off:],
                                    vs[st // NSUB][:, st % NSUB, :],
                                    pT2[:, off:],
                                    start=(st == 0), stop=(st == n_st - 1),
                                    skip_group_check=True)
                                nc.tensor.matmul(
                                    l_ps[:, off:], ones_sq, pT2[:, off:],
                                    start=(st == 0), stop=(st == n_st - 1),
                                    skip_group_check=True)

                            prev = scores_block(0)
                            for st in range(1, n_st):
                                cur = scores_block(st)
                                av_block(st - 1, prev)
                                prev = cur
                            av_block(n_st - 1, prev)
                            nc.scalar.activation(av_all[:, h, :], av_ps, Copy)
                            nc.scalar.activation(l_all[:, h, :], l_ps, Copy)
                        # 1/l as exp(-ln(l)) on ACT: DVE reciprocal is ~8
                        # cycles/elem (4us per [128,512]) and would gate the
                        # per-head chain.
                        lg = avlpool.tile([H, NHC, TCH], f32, tag="lg")
                        nc.scalar.activation(lg, l_all, Ln)
                        rl = avlpool.tile([H, NHC, TCH], f32, tag="rl")
                        nc.scalar.activation(rl, lg, Exp, scale=-1.0)
                        for h in range(NHC):
                            nc.vector.tensor_mul(outT_sb[:, h, :],
                                                 av_all[:, h, :], rl[:, h, :])
                        pending = (b, c, outT_sb)
            with ExitStack() as s3:
                po = s3.enter_context(
                    tc.tile_pool(name="po3", bufs=2, space="PSUM"))
                emit_oproj(pending, po)

    nc.compile()
    return nc


_NC_CACHE = None


def kernel(x, wq, wk, wv, wo, positions):
    global _NC_CACHE
    import ml_dtypes
    from concourse.bass_utils import run_bass_kernel_spmd

    bf16 = ml_dtypes.bfloat16
    x = np.asarray(x, dtype=np.float32)
    wq = np.asarray(wq, dtype=np.float32)
    wk = np.asarray(wk, dtype=np.float32)
    wv = np.asarray(wv, dtype=np.float32)
    wo = np.asarray(wo, dtype=np.float32)
    positions = np.asarray(positions)

    xT = np.ascontiguousarray(x.reshape(TOK, D).T).astype(bf16)
    # rope tables, [H, B*T]: duplicated across partition halves; sin's
    # first half negated (see kernel docstring).
    fraction = 2.0 * np.arange(HH, dtype=np.float32) / H
    timescale = (THETA ** fraction).astype(np.float32)
    pos = positions.reshape(TOK).astype(np.float32)
    sinusoid = pos[None, :] / timescale[:, None]
    cos_h = np.cos(sinusoid).astype(np.float32)
    sin_h = np.sin(sinusoid).astype(np.float32)
    cosT = np.ascontiguousarray(np.concatenate([cos_h, cos_h], axis=0))
    sinST = np.ascontiguousarray(np.concatenate([-sin_h, sin_h], axis=0))

    if _NC_CACHE is None:
        _NC_CACHE = _build_bass()
    nc = _NC_CACHE

    in_maps = []
    for c in range(NCORES):
        in_maps.append({
            "xT": xT,
            "wq": np.ascontiguousarray(wq[c * NHC:(c + 1) * NHC]).astype(bf16),
            "wk": np.ascontiguousarray(wk[c]).astype(bf16),
            "wv": np.ascontiguousarray(wv[c]).astype(bf16),
            "wo": np.ascontiguousarray(wo[c * NHC:(c + 1) * NHC]).astype(bf16),
            "cosT": cosT,
            "sinST": sinST,
        })

    trace = os.environ.get("BASS_KERNEL_TRACE", "0") == "1"
    res = run_bass_kernel_spmd(nc, in_maps, list(range(NCORES)), trace=trace)
    global LAST_RESULTS
    LAST_RESULTS = res
    out = np.zeros((TOK, D), dtype=np.float32)
    for c in range(NCORES):
        out += res.results[c]["o_part"].astype(np.float32)
    return out.reshape(B, T, D)


LAST_RESULTS = None


# revision 14
# speedup vs baseline: 1.2538x; 1.0247x over previous
"""GQA causal-attention prefill kernel for Trainium2, tensor-parallel over 8 NeuronCores.

v2: all-bf16 datapath, fused single-stream schedule.

Sharding: head-parallel. Core c gets q heads [4c, 4c+4), kv head c, and the
matching wo slice; each core computes a full-shape partial output
o_part = attn(heads of c) @ wo_c and the host sums the 8 partials.

Per-core schedule (one PE stream, no phase barriers):
  for b, c:   PROJ(b,c) k-sweep -> OPROJ(prev chunk) -> ATTN(b,c)
The o-projection of the previous chunk fills the PE while the current
chunk's rope evictions run on DVE, so the PE never waits on the rope tail.

Layouts:
  - xT [D, B*T] bf16; projections contract D on the partition dim:
    qT/kT/vT[h] = w[h].T @ xT -> [H=128, tokens], 6 PSUM groups per chunk.
  - RoPE on eviction, all on DVE reading PSUM directly (4 ops/group):
      tmp[0:64]  = psum[64:128] * sinS[0:64]   (sinS first half = -sin)
      tmp[64:]   = psum[0:64]   * sinS[64:]    (second half = +sin)
      tmp2       = psum * cos
      dst(bf16)  = tmp + tmp2
  - scores transposed (sT[s,t] = kT_tile.T @ qT), exp fused into PSUM
    eviction on ACT (bf16 out); causal diagonal blocks are column-trimmed
    (ap = 512-128j) and masked by a 0/1 wedge multiply on DVE.
  - softmax denominator: lbc[m,t] += ones[128,128].T @ p  (all-ones
    stationary broadcasts the column sum to all 128 partitions, so the
    reciprocal needs no separate broadcast matmul).
  - normalize fused into av eviction: outT[:,h,:] = av_psum * recip(lbc).
  - o-projection contracts (h,H) against natural wo; evictions alternate
    ACT/DVE; output stays f32.
"""

import os
import sys

sys.path.insert(0, "/opt/trn_rl_repo")

import numpy as np

B = 2
T = 2048
TOK = B * T
D = 4096
NQ = 32
NKV = 8
H = 128
HH = H // 2
THETA = 10000.0
NCORES = 8
NHC = NQ // NCORES          # q heads per core (4)
KPC = D // H                # contraction chunks of 128 over D (32)
TCH = 512                   # token chunk
NTCH = T // TCH             # 4 token chunks per batch
NSUB = TCH // H             # 4 128-wide subtiles per chunk
C_SM = 1.0 / np.sqrt(H)     # softmax scale


def _build_bass():
    import concourse.bacc as bacc
    import concourse.mybir as mybir
    import concourse.tile as tile
    from concourse.masks import make_identity
    from contextlib import ExitStack

    f32 = mybir.dt.float32
    bf16 = mybir.dt.bfloat16
    Exp = mybir.ActivationFunctionType.Exp
    Ln = mybir.ActivationFunctionType.Ln
    Copy = mybir.ActivationFunctionType.Copy

    nc = bacc.Bacc("TRN2", target_bir_lowering=False, debug=False,
                   num_devices=NCORES)

    xT = nc.declare_dram_parameter("xT", [D, TOK], bf16, isOutput=False)
    wq = nc.declare_dram_parameter("wq", [NHC, D, H], bf16, isOutput=False)
    wk = nc.declare_dram_parameter("wk", [D, H], bf16, isOutput=False)
    wv = nc.declare_dram_parameter("wv", [D, H], bf16, isOutput=False)
    wo = nc.declare_dram_parameter("wo", [NHC, H, D], bf16, isOutput=False)
    # rope tables duplicated across both partition halves; sinS's first
    # half is negated so rope reduces to dst = psum*cos + swap(psum)*sinS.
    cosT = nc.declare_dram_parameter("cosT", [H, TOK], f32, isOutput=False)
    sinST = nc.declare_dram_parameter("sinST", [H, TOK], f32, isOutput=False)
    o_part = nc.declare_dram_parameter("o_part", [TOK, D], bf16,
                                      isOutput=True)

    with tile.TileContext(nc) as tc:
        with ExitStack() as top:
            consts = top.enter_context(tc.tile_pool(name="consts", bufs=1))
            identity = consts.tile([H, H], bf16)
            make_identity(nc, identity)
            ones_sq = consts.tile([H, H], bf16, tag="ones")
            nc.vector.memset(ones_sq, 1.0)
            # 0/1 causal wedge masks: mask[j][s, t] = 1 iff (t - s - 128j) >= 0
            masks = []
            for j in range(NSUB):
                m = consts.tile([H, TCH], bf16, tag=f"mask{j}",
                                name=f"mask{j}")
                nc.vector.memset(m, 1.0)
                nc.gpsimd.affine_select(
                    out=m, in_=m,
                    compare_op=mybir.AluOpType.is_ge,
                    fill=0.0,
                    base=-H * j,
                    pattern=[[1, TCH]],
                    channel_multiplier=-1,
                )
                masks.append(m)

            # ---- persistent weights / tables ----
            wpool = top.enter_context(tc.tile_pool(name="wpool", bufs=1))
            wq_src = wq.rearrange("h (c p) m -> p h c m", p=H)
            wqs = [wpool.tile([H, KPC, H], bf16, tag=f"wq{i}",
                              name=f"wq{i}") for i in range(NHC)]
            wk_sb = wpool.tile([H, KPC, H], bf16, tag="wk")
            wk_src = wk.rearrange("(c p) m -> p c m", p=H)
            wv_sb = wpool.tile([H, KPC, H], bf16, tag="wv")
            wv_src = wv.rearrange("(c p) m -> p c m", p=H)
            # k-major emission: the first sweep needs sub-chunk 0 of all six
            # weight tensors, not all of wq0 -- keeps the front of the
            # kernel from DMA-starving behind 14MB of weight loads.
            for c8 in range(4):
                sl = slice(c8 * 8, (c8 + 1) * 8)
                for i in range(NHC):
                    nc.sync.dma_start(out=wqs[i][:, sl, :],
                                      in_=wq_src[:, i, sl, :])
                nc.sync.dma_start(out=wk_sb[:, sl, :], in_=wk_src[:, sl, :])
                nc.sync.dma_start(out=wv_sb[:, sl, :], in_=wv_src[:, sl, :])
            cos_sb = wpool.tile([H, TOK], f32, tag="cos")
            sin_sb = wpool.tile([H, TOK], f32, tag="sin")
            for c4 in range(4):
                sl = slice(c4 * 1024, (c4 + 1) * 1024)
                nc.sync.dma_start(out=cos_sb[:, sl], in_=cosT[:, sl])
                nc.sync.dma_start(out=sin_sb[:, sl], in_=sinST[:, sl])
            wo_sb = wpool.tile([H, NHC, D], bf16, tag="wo")
            wo_src = wo.rearrange("h p d -> p h d")
            for dc8 in range(8):
                sl = slice(dc8 * TCH, (dc8 + 1) * TCH)
                nc.sync.dma_start(out=wo_sb[:, :, sl], in_=wo_src[:, :, sl])

            # ---- persistent activations ----
            act = top.enter_context(tc.tile_pool(name="act", bufs=1))
            qTs = [act.tile([H, NHC, TCH], bf16, tag=f"qT{i}",
                            name=f"qT{i}") for i in range(NTCH)]
            kTs = [act.tile([H, TCH], bf16, tag=f"kT{i}",
                            name=f"kT{i}") for i in range(NTCH)]
            vs = [act.tile([H, NSUB, H], bf16, tag=f"v{i}",
                           name=f"v{i}") for i in range(NTCH)]

            xpool = top.enter_context(tc.tile_pool(name="xpool", bufs=12))
            rtmp = top.enter_context(tc.tile_pool(name="rtmp", bufs=2))
            vstg = top.enter_context(tc.tile_pool(name="vstg", bufs=2))
            ppool = top.enter_context(tc.tile_pool(name="ppool", bufs=4))
            pdiag = top.enter_context(tc.tile_pool(name="pdiag", bufs=2))
            avlpool = top.enter_context(tc.tile_pool(name="avlpool", bufs=1))
            otpool = top.enter_context(tc.tile_pool(name="otpool", bufs=2))
            opool = top.enter_context(tc.tile_pool(name="opool", bufs=7))

            def rope(psum, dst, cs, sn):
                tmp = rtmp.tile([H, TCH], f32, tag="rt")
                tmp2 = rtmp.tile([H, TCH], f32, tag="rt2")
                nc.vector.tensor_mul(tmp[0:HH, :], psum[HH:H, :], sn[0:HH, :])
                nc.vector.tensor_mul(tmp[HH:H, :], psum[0:HH, :], sn[HH:H, :])
                nc.vector.tensor_mul(tmp2, psum, cs)
                nc.vector.tensor_add(dst, tmp, tmp2)

            def emit_oproj(pend, po):
                pb, pc, outT = pend
                for u in range(NSUB):
                    trow = pb * T + pc * TCH + u * H
                    for dc in range(D // TCH):
                        ops = po.tile([H, TCH], f32, tag="o")
                        for h in range(NHC):
                            nc.tensor.matmul(
                                ops,
                                outT[:, h, u * H:(u + 1) * H],
                                wo_sb[:, h, dc * TCH:(dc + 1) * TCH],
                                start=(h == 0), stop=(h == NHC - 1),
                                skip_group_check=True)
                        o_sb = opool.tile([H, TCH], bf16, tag="osb")
                        nc.scalar.activation(o_sb, ops, Copy)
                        nc.sync.dma_start(
                            out=o_part[trow:trow + H,
                                       dc * TCH:(dc + 1) * TCH],
                            in_=o_sb)

            pending = None
            for b in range(B):
                tb = b * T
                for c in range(NTCH):
                    t0 = tb + c * TCH
                    cs = cos_sb[:, t0:t0 + TCH]
                    sn = sin_sb[:, t0:t0 + TCH]
                    with ExitStack() as s1:
                        pj = s1.enter_context(
                            tc.tile_pool(name="pj", bufs=1, space="PSUM"))
                        po = s1.enter_context(
                            tc.tile_pool(name="po", bufs=2, space="PSUM"))
                        g = [pj.tile([H, TCH], f32, tag=f"g{i}",
                                     name=f"g{i}") for i in range(6)]
                        for k in range(KPC):
                            x_t = xpool.tile([H, TCH], bf16, tag="x")
                            nc.sync.dma_start(
                                out=x_t,
                                in_=xT[k * H:(k + 1) * H, t0:t0 + TCH])
                            lhs = [wqs[0][:, k, :], wqs[1][:, k, :],
                                   wqs[2][:, k, :], wqs[3][:, k, :],
                                   wk_sb[:, k, :], wv_sb[:, k, :]]
                            for i in range(6):
                                nc.tensor.matmul(
                                    g[i], lhs[i], x_t,
                                    start=(k == 0), stop=(k == KPC - 1),
                                    skip_group_check=True)
                        # evictions first in ACT/DVE queue order: v staging
                        # (ACT) and rope (DVE) so the attention inputs are
                        # ready early; the o-projection's PE work then
                        # covers them, with its o_sb evictions (ACT) queued
                        # behind and paced by its own matmuls.
                        vt_stage = vstg.tile([H, TCH], bf16, tag="vstage")
                        nc.scalar.activation(vt_stage, g[5], Copy)
                        rope(g[0], qTs[c][:, 0, :], cs, sn)
                        rope(g[4], kTs[c], cs, sn)
                        rope(g[1], qTs[c][:, 1, :], cs, sn)
                        rope(g[2], qTs[c][:, 2, :], cs, sn)
                        rope(g[3], qTs[c][:, 3, :], cs, sn)
                        if pending is not None:
                            emit_oproj(pending, po)
                            pending = None
                    with ExitStack() as s2:
                        pt = s2.enter_context(
                            tc.tile_pool(name="pt", bufs=2, space="PSUM"))
                        ps_s = s2.enter_context(
                            tc.tile_pool(name="ps_s", bufs=2, space="PSUM"))
                        ps_av = s2.enter_context(
                            tc.tile_pool(name="ps_av", bufs=2, space="PSUM"))
                        ps_l = s2.enter_context(
                            tc.tile_pool(name="ps_l", bufs=2, space="PSUM"))
                        for j in range(NSUB):
                            tp = pt.tile([H, H], bf16, tag="vtp")
                            nc.tensor.transpose(
                                tp, vt_stage[:, j * H:(j + 1) * H], identity)
                            nc.scalar.activation(vs[c][:, j, :], tp, Copy)
                        n_st = (c + 1) * NSUB
                        outT_sb = otpool.tile([H, NHC, TCH], bf16, tag="outT")
                        # per-head av/l leave PSUM via plain ACT copies (no
                        # activation-table switch); the Ln/Exp reciprocal is
                        # batched once per chunk so the Exp<->Ln table reload
                        # (1.3us each) happens 2x per chunk instead of 2x
                        # per head, off the per-head critical path.
                        av_all = avlpool.tile([H, NHC, TCH], bf16, tag="ava")
                        l_all = avlpool.tile([H, NHC, TCH], bf16, tag="la")
                        for h in range(NHC):
                            av_ps = ps_av.tile([H, TCH], f32, tag="av")
                            l_ps = ps_l.tile([H, TCH], f32, tag="lbc")

                            def scores_block(st):
                                j = st - c * NSUB
                                off = H * j if j > 0 else 0
                                kt = kTs[st // NSUB][
                                    :, (st % NSUB) * H:(st % NSUB + 1) * H]
                                sps = ps_s.tile([H, TCH], f32, tag="s")
                                nc.tensor.matmul(
                                    sps[:, off:], kt, qTs[c][:, h, off:],
                                    start=True, stop=True)
                                pT2 = ppool.tile([H, TCH], bf16, tag="p2")
                                if j >= 0:
                                    pT = pdiag.tile([H, TCH], bf16, tag="pd")
                                    nc.scalar.activation(
                                        pT[:, off:], sps[:, off:], Exp,
                                        scale=C_SM)
                                    nc.vector.tensor_mul(
                                        pT2[:, off:], pT[:, off:],
                                        masks[j][:, off:])
                                else:
                                    nc.scalar.activation(
                                        pT2, sps, Exp, scale=C_SM)
                                return pT2

                            def av_block(st, pT2):
                                j = st - c * NSUB
                                off = H * j if j > 0 else 0
                                nc.tensor.matmul(
                                    av_ps[:, off:],
                                    vs[st // NSUB][:, st % NSUB, :],
                                    pT2[:, off:],
                                    start=(st == 0), stop=(st == n_st - 1),
                                    skip_group_check=True)
                                nc.tensor.matmul(
                                    l_ps[:, off:], ones_sq, pT2[:, off:],
                                    start=(st == 0), stop=(st == n_st - 1),
                                    skip_group_check=True)

                            prev = scores_block(0)
                            for st in range(1, n_st):
                                cur = scores_block(st)
                                av_block(st - 1, prev)
                                prev = cur
                            av_block(n_st - 1, prev)
                            nc.scalar.activation(av_all[:, h, :], av_ps, Copy)
                            nc.scalar.activation(l_all[:, h, :], l_ps, Copy)
                        # 1/l as exp(-ln(l)) on ACT: DVE reciprocal is ~8
                        # cycles/elem (4us per [128,512]) and would gate the
                        # per-head chain.
                        lg = avlpool.tile([H, NHC, TCH], f32, tag="lg")
                        nc.scalar.activation(lg, l_all, Ln)
                        rl = avlpool.tile([H, NHC, TCH], f32, tag="rl")
                        nc.scalar.activation(rl, lg, Exp, scale=-1.0)
                        for h in range(NHC):
                            nc.vector.tensor_mul(outT_sb[:, h, :],
                                                 av_all[:, h, :], rl[:, h, :])
                        pending = (b, c, outT_sb)
            with ExitStack() as s3:
                po = s3.enter_context(
                    tc.tile_pool(name="po3", bufs=2, space="PSUM"))
                emit_oproj(pending, po)

    nc.compile()
    return nc


_NC_CACHE = None


def kernel(x, wq, wk, wv, wo, positions):
    global _NC_CACHE
    import ml_dtypes
    from concourse.bass_utils import run_bass_kernel_spmd

    bf16 = ml_dtypes.bfloat16
    x = np.asarray(x, dtype=np.float32)
    wq = np.asarray(wq, dtype=np.float32)
    wk = np.asarray(wk, dtype=np.float32)
    wv = np.asarray(wv, dtype=np.float32)
    wo = np.asarray(wo, dtype=np.float32)
    positions = np.asarray(positions)

    xT = np.ascontiguousarray(x.reshape(TOK, D).T).astype(bf16)
    # rope tables, [H, B*T]: duplicated across partition halves; sin's
    # first half negated (see kernel docstring).
    fraction = 2.0 * np.arange(HH, dtype=np.float32) / H
    timescale = (THETA ** fraction).astype(np.float32)
    pos = positions.reshape(TOK).astype(np.float32)
    sinusoid = pos[None, :] / timescale[:, None]
    cos_h = np.cos(sinusoid).astype(np.float32)
    sin_h = np.sin(sinusoid).astype(np.float32)
    cosT = np.ascontiguousarray(np.concatenate([cos_h, cos_h], axis=0))
    sinST = np.ascontiguousarray(np.concatenate([-sin_h, sin_h], axis=0))

    if _NC_CACHE is None:
        _NC_CACHE = _build_bass()
    nc = _NC_CACHE

    in_maps = []
    for c in range(NCORES):
        in_maps.append({
            "xT": xT,
            "wq": np.ascontiguousarray(wq[c * NHC:(c + 1) * NHC]).astype(bf16),
            "wk": np.ascontiguousarray(wk[c]).astype(bf16),
            "wv": np.ascontiguousarray(wv[c]).astype(bf16),
            "wo": np.ascontiguousarray(wo[c * NHC:(c + 1) * NHC]).astype(bf16),
            "cosT": cosT,
            "sinST": sinST,
        })

    trace = os.environ.get("BASS_KERNEL_TRACE", "0") == "1"
    res = run_bass_kernel_spmd(nc, in_maps, list(range(NCORES)), trace=trace)
    global LAST_RESULTS
    LAST_RESULTS = res
    out = np.zeros((TOK, D), dtype=np.float32)
    for c in range(NCORES):
        out += res.results[c]["o_part"].astype(np.float32)
    return out.reshape(B, T, D)


LAST_RESULTS = None


# revision 17
# speedup vs baseline: 1.2895x; 1.0285x over previous
"""GQA causal-attention prefill kernel for Trainium2, tensor-parallel over 8 NeuronCores.

v2: all-bf16 datapath, fused single-stream schedule.

Sharding: head-parallel. Core c gets q heads [4c, 4c+4), kv head c, and the
matching wo slice; each core computes a full-shape partial output
o_part = attn(heads of c) @ wo_c and the host sums the 8 partials.

Per-core schedule (one PE stream, no phase barriers):
  for b, c:   PROJ(b,c) k-sweep -> OPROJ(prev chunk) -> ATTN(b,c)
The o-projection of the previous chunk fills the PE while the current
chunk's rope evictions run on DVE, so the PE never waits on the rope tail.

Layouts:
  - xT [D, B*T] bf16; projections contract D on the partition dim:
    qT/kT/vT[h] = w[h].T @ xT -> [H=128, tokens], 6 PSUM groups per chunk.
  - RoPE on eviction, all on DVE reading PSUM directly (4 ops/group):
      tmp[0:64]  = psum[64:128] * sinS[0:64]   (sinS first half = -sin)
      tmp[64:]   = psum[0:64]   * sinS[64:]    (second half = +sin)
      tmp2       = psum * cos
      dst(bf16)  = tmp + tmp2
  - scores transposed (sT[s,t] = kT_tile.T @ qT), exp fused into PSUM
    eviction on ACT (bf16 out); causal diagonal blocks are column-trimmed
    (ap = 512-128j) and masked by a 0/1 wedge multiply on DVE.
  - softmax denominator: lbc[m,t] += ones[128,128].T @ p  (all-ones
    stationary broadcasts the column sum to all 128 partitions, so the
    reciprocal needs no separate broadcast matmul).
  - normalize fused into av eviction: outT[:,h,:] = av_psum * recip(lbc).
  - o-projection contracts (h,H) against natural wo; evictions alternate
    ACT/DVE; output stays f32.
"""

import os
import sys

sys.path.insert(0, "/opt/trn_rl_repo")

import numpy as np

B = 2
T = 2048
TOK = B * T
D = 4096
NQ = 32
NKV = 8
H = 128
HH = H // 2
THETA = 10000.0
NCORES = 8
NHC = NQ // NCORES          # q heads per core (4)
KPC = D // H                # contraction chunks of 128 over D (32)
TCH = 512                   # token chunk
NTCH = T // TCH             # 4 token chunks per batch
NSUB = TCH // H             # 4 128-wide subtiles per chunk
C_SM = 1.0 / np.sqrt(H)     # softmax scale


def _build_bass():
    import concourse.bacc as bacc
    import concourse.mybir as mybir
    import concourse.tile as tile
    from concourse.masks import make_identity
    from contextlib import ExitStack

    f32 = mybir.dt.float32
    bf16 = mybir.dt.bfloat16
    Exp = mybir.ActivationFunctionType.Exp
    Ln = mybir.ActivationFunctionType.Ln
    Copy = mybir.ActivationFunctionType.Copy

    nc = bacc.Bacc("TRN2", target_bir_lowering=False, debug=False,
                   num_devices=NCORES)

    xT = nc.declare_dram_parameter("xT", [D, TOK], bf16, isOutput=False)
    wq = nc.declare_dram_parameter("wq", [NHC, D, H], bf16, isOutput=False)
    wk = nc.declare_dram_parameter("wk", [D, H], bf16, isOutput=False)
    wv = nc.declare_dram_parameter("wv", [D, H], bf16, isOutput=False)
    wo = nc.declare_dram_parameter("wo", [NHC, H, D], bf16, isOutput=False)
    # rope tables duplicated across both partition halves; sinS's first
    # half is negated so rope reduces to dst = psum*cos + swap(psum)*sinS.
    cosT = nc.declare_dram_parameter("cosT", [H, TOK], f32, isOutput=False)
    sinST = nc.declare_dram_parameter("sinST", [H, TOK], f32, isOutput=False)
    o_part = nc.declare_dram_parameter("o_part", [TOK, D], bf16,
                                      isOutput=True)

    with tile.TileContext(nc) as tc:
        with ExitStack() as top:
            consts = top.enter_context(tc.tile_pool(name="consts", bufs=1))
            identity = consts.tile([H, H], bf16)
            make_identity(nc, identity)
            ones_sq = consts.tile([H, H], bf16, tag="ones")
            nc.vector.memset(ones_sq, 1.0)
            # 0/1 causal wedge masks: mask[j][s, t] = 1 iff (t - s - 128j) >= 0
            masks = []
            for j in range(NSUB):
                m = consts.tile([H, TCH], bf16, tag=f"mask{j}",
                                name=f"mask{j}")
                nc.vector.memset(m, 1.0)
                nc.gpsimd.affine_select(
                    out=m, in_=m,
                    compare_op=mybir.AluOpType.is_ge,
                    fill=0.0,
                    base=-H * j,
                    pattern=[[1, TCH]],
                    channel_multiplier=-1,
                )
                masks.append(m)

            # ---- persistent weights / tables ----
            wpool = top.enter_context(tc.tile_pool(name="wpool", bufs=1))
            wq_src = wq.rearrange("h (c p) m -> p h c m", p=H)
            wqs = [wpool.tile([H, KPC, H], bf16, tag=f"wq{i}",
                              name=f"wq{i}") for i in range(NHC)]
            wk_sb = wpool.tile([H, KPC, H], bf16, tag="wk")
            wk_src = wk.rearrange("(c p) m -> p c m", p=H)
            wv_sb = wpool.tile([H, KPC, H], bf16, tag="wv")
            wv_src = wv.rearrange("(c p) m -> p c m", p=H)
            # k-major emission: the first sweep needs sub-chunk 0 of all six
            # weight tensors, not all of wq0 -- keeps the front of the
            # kernel from DMA-starving behind 14MB of weight loads.
            for c8 in range(4):
                sl = slice(c8 * 8, (c8 + 1) * 8)
                for i in range(NHC):
                    nc.sync.dma_start(out=wqs[i][:, sl, :],
                                      in_=wq_src[:, i, sl, :])
                nc.sync.dma_start(out=wk_sb[:, sl, :], in_=wk_src[:, sl, :])
                nc.sync.dma_start(out=wv_sb[:, sl, :], in_=wv_src[:, sl, :])
            # cos/sin and wo tiles are declared here but their DMAs are
            # emitted inside the first chunk's k-loop: the first sweep's x
            # stream must not queue behind 8MB of tables it doesn't need
            # until the first rope (~40us) / first o-projection (~90us).
            cos_sb = wpool.tile([H, TOK], f32, tag="cos")
            sin_sb = wpool.tile([H, TOK], f32, tag="sin")
            wo_sb = wpool.tile([H, NHC, D], bf16, tag="wo")
            wo_src = wo.rearrange("h p d -> p h d")

            def emit_table_dmas():
                for c4 in range(4):
                    sl = slice(c4 * 1024, (c4 + 1) * 1024)
                    nc.sync.dma_start(out=cos_sb[:, sl], in_=cosT[:, sl])
                    nc.sync.dma_start(out=sin_sb[:, sl], in_=sinST[:, sl])

            def emit_wo_dmas():
                for dc8 in range(8):
                    sl = slice(dc8 * TCH, (dc8 + 1) * TCH)
                    nc.sync.dma_start(out=wo_sb[:, :, sl],
                                      in_=wo_src[:, :, sl])

            # ---- persistent activations ----
            act = top.enter_context(tc.tile_pool(name="act", bufs=1))
            qTs = [act.tile([H, NHC, TCH], bf16, tag=f"qT{i}",
                            name=f"qT{i}") for i in range(NTCH)]
            kTs = [act.tile([H, TCH], bf16, tag=f"kT{i}",
                            name=f"kT{i}") for i in range(NTCH)]
            vs = [act.tile([H, NSUB, H], bf16, tag=f"v{i}",
                           name=f"v{i}") for i in range(NTCH)]

            xpool = top.enter_context(tc.tile_pool(name="xpool", bufs=12))
            rtmp = top.enter_context(tc.tile_pool(name="rtmp", bufs=2))
            vstg = top.enter_context(tc.tile_pool(name="vstg", bufs=2))
            ppool = top.enter_context(tc.tile_pool(name="ppool", bufs=4))
            pdiag = top.enter_context(tc.tile_pool(name="pdiag", bufs=2))
            avlpool = top.enter_context(tc.tile_pool(name="avlpool", bufs=1))
            otpool = top.enter_context(tc.tile_pool(name="otpool", bufs=2))
            opool = top.enter_context(tc.tile_pool(name="opool", bufs=7))

            def rope(psum, dst, cs, sn):
                tmp = rtmp.tile([H, TCH], f32, tag="rt")
                tmp2 = rtmp.tile([H, TCH], f32, tag="rt2")
                nc.vector.tensor_mul(tmp[0:HH, :], psum[HH:H, :], sn[0:HH, :])
                nc.vector.tensor_mul(tmp[HH:H, :], psum[0:HH, :], sn[HH:H, :])
                nc.vector.tensor_mul(tmp2, psum, cs)
                nc.vector.tensor_add(dst, tmp, tmp2)

            def emit_oproj(pend, po):
                pb, pc, outT = pend
                for u in range(NSUB):
                    trow = pb * T + pc * TCH + u * H
                    for dc in range(D // TCH):
                        ops = po.tile([H, TCH], f32, tag="o")
                        for h in range(NHC):
                            nc.tensor.matmul(
                                ops,
                                outT[:, h, u * H:(u + 1) * H],
                                wo_sb[:, h, dc * TCH:(dc + 1) * TCH],
                                start=(h == 0), stop=(h == NHC - 1),
                                skip_group_check=True)
                        o_sb = opool.tile([H, TCH], bf16, tag="osb")
                        nc.scalar.activation(o_sb, ops, Copy)
                        nc.sync.dma_start(
                            out=o_part[trow:trow + H,
                                       dc * TCH:(dc + 1) * TCH],
                            in_=o_sb)

            pending = None
            for b in range(B):
                tb = b * T
                for c in range(NTCH):
                    t0 = tb + c * TCH
                    cs = cos_sb[:, t0:t0 + TCH]
                    sn = sin_sb[:, t0:t0 + TCH]
                    with ExitStack() as s1:
                        pj = s1.enter_context(
                            tc.tile_pool(name="pj", bufs=1, space="PSUM"))
                        po = s1.enter_context(
                            tc.tile_pool(name="po", bufs=2, space="PSUM"))
                        g = [pj.tile([H, TCH], f32, tag=f"g{i}",
                                     name=f"g{i}") for i in range(6)]
                        for k in range(KPC):
                            x_t = xpool.tile([H, TCH], bf16, tag="x")
                            nc.sync.dma_start(
                                out=x_t,
                                in_=xT[k * H:(k + 1) * H, t0:t0 + TCH])
                            lhs = [wqs[0][:, k, :], wqs[1][:, k, :],
                                   wqs[2][:, k, :], wqs[3][:, k, :],
                                   wk_sb[:, k, :], wv_sb[:, k, :]]
                            for i in range(6):
                                nc.tensor.matmul(
                                    g[i], lhs[i], x_t,
                                    start=(k == 0), stop=(k == KPC - 1),
                                    skip_group_check=True)
                            if b == 0 and c == 0:
                                if k == 1:
                                    emit_table_dmas()
                                elif k == 16:
                                    emit_wo_dmas()
                        # evictions first in ACT/DVE queue order: v staging
                        # (ACT) and rope (DVE) so the attention inputs are
                        # ready early; the o-projection's PE work then
                        # covers them, with its o_sb evictions (ACT) queued
                        # behind and paced by its own matmuls.
                        vt_stage = vstg.tile([H, TCH], bf16, tag="vstage")
                        nc.scalar.activation(vt_stage, g[5], Copy)
                        rope(g[0], qTs[c][:, 0, :], cs, sn)
                        rope(g[4], kTs[c], cs, sn)
                        rope(g[1], qTs[c][:, 1, :], cs, sn)
                        rope(g[2], qTs[c][:, 2, :], cs, sn)
                        rope(g[3], qTs[c][:, 3, :], cs, sn)
                        if pending is not None:
                            emit_oproj(pending, po)
                            pending = None
                    with ExitStack() as s2:
                        pt = s2.enter_context(
                            tc.tile_pool(name="pt", bufs=2, space="PSUM"))
                        ps_s = s2.enter_context(
                            tc.tile_pool(name="ps_s", bufs=2, space="PSUM"))
                        ps_av = s2.enter_context(
                            tc.tile_pool(name="ps_av", bufs=2, space="PSUM"))
                        ps_l = s2.enter_context(
                            tc.tile_pool(name="ps_l", bufs=2, space="PSUM"))
                        for j in range(NSUB):
                            tp = pt.tile([H, H], bf16, tag="vtp")
                            nc.tensor.transpose(
                                tp, vt_stage[:, j * H:(j + 1) * H], identity)
                            nc.scalar.activation(vs[c][:, j, :], tp, Copy)
                        n_st = (c + 1) * NSUB
                        outT_sb = otpool.tile([H, NHC, TCH], bf16, tag="outT")
                        # per-head av/l leave PSUM via plain ACT copies (no
                        # activation-table switch); the Ln/Exp reciprocal is
                        # batched once per chunk so the Exp<->Ln table reload
                        # (1.3us each) happens 2x per chunk instead of 2x
                        # per head, off the per-head critical path.
                        av_all = avlpool.tile([H, NHC, TCH], bf16, tag="ava")
                        l_all = avlpool.tile([H, NHC, TCH], bf16, tag="la")
                        for h in range(NHC):
                            av_ps = ps_av.tile([H, TCH], f32, tag="av")
                            l_ps = ps_l.tile([H, TCH], f32, tag="lbc")

                            def scores_block(st):
                                j = st - c * NSUB
                                off = H * j if j > 0 else 0
                                kt = kTs[st // NSUB][
                                    :, (st % NSUB) * H:(st % NSUB + 1) * H]
                                sps = ps_s.tile([H, TCH], f32, tag="s")
                                nc.tensor.matmul(
                                    sps[:, off:], kt, qTs[c][:, h, off:],
                                    start=True, stop=True)
                                pT2 = ppool.tile([H, TCH], bf16, tag="p2")
                                if j >= 0:
                                    pT = pdiag.tile([H, TCH], bf16, tag="pd")
                                    nc.scalar.activation(
                                        pT[:, off:], sps[:, off:], Exp,
                                        scale=C_SM)
                                    nc.vector.tensor_mul(
                                        pT2[:, off:], pT[:, off:],
                                        masks[j][:, off:])
                                else:
                                    nc.scalar.activation(
                                        pT2, sps, Exp, scale=C_SM)
                                return pT2

                            def av_block(st, pT2):
                                j = st - c * NSUB
                                off = H * j if j > 0 else 0
                                nc.tensor.matmul(
                                    av_ps[:, off:],
                                    vs[st // NSUB][:, st % NSUB, :],
                                    pT2[:, off:],
                                    start=(st == 0), stop=(st == n_st - 1),
                                    skip_group_check=True)
                                nc.tensor.matmul(
                                    l_ps[:, off:], ones_sq, pT2[:, off:],
                                    start=(st == 0), stop=(st == n_st - 1),
                                    skip_group_check=True)

                            prev = scores_block(0)
                            for st in range(1, n_st):
                                cur = scores_block(st)
                                av_block(st - 1, prev)
                                prev = cur
                            av_block(n_st - 1, prev)
                            # evict av/l on DVE: ACT copies here would
                            # interleave into the exp stream and stall the
                            # next head's first av matmul ~1us.
                            nc.vector.tensor_copy(av_all[:, h, :], av_ps)
                            nc.vector.tensor_copy(l_all[:, h, :], l_ps)
                        # 1/l as exp(-ln(l)) on ACT: DVE reciprocal is ~8
                        # cycles/elem (4us per [128,512]) and would gate the
                        # per-head chain.
                        lg = avlpool.tile([H, NHC, TCH], f32, tag="lg")
                        nc.scalar.activation(lg, l_all, Ln)
                        rl = avlpool.tile([H, NHC, TCH], f32, tag="rl")
                        nc.scalar.activation(rl, lg, Exp, scale=-1.0)
                        for h in range(NHC):
                            nc.vector.tensor_mul(outT_sb[:, h, :],
                                                 av_all[:, h, :], rl[:, h, :])
                        pending = (b, c, outT_sb)
            with ExitStack() as s3:
                po = s3.enter_context(
                    tc.tile_pool(name="po3", bufs=2, space="PSUM"))
                emit_oproj(pending, po)

    nc.compile()
    return nc


_NC_CACHE = None


def kernel(x, wq, wk, wv, wo, positions):
    global _NC_CACHE
    import ml_dtypes
    from concourse.bass_utils import run_bass_kernel_spmd

    bf16 = ml_dtypes.bfloat16
    x = np.asarray(x, dtype=np.float32)
    wq = np.asarray(wq, dtype=np.float32)
    wk = np.asarray(wk, dtype=np.float32)
    wv = np.asarray(wv, dtype=np.float32)
    wo = np.asarray(wo, dtype=np.float32)
    positions = np.asarray(positions)

    xT = np.ascontiguousarray(x.reshape(TOK, D).T).astype(bf16)
    # rope tables, [H, B*T]: duplicated across partition halves; sin's
    # first half negated (see kernel docstring).
    fraction = 2.0 * np.arange(HH, dtype=np.float32) / H
    timescale = (THETA ** fraction).astype(np.float32)
    pos = positions.reshape(TOK).astype(np.float32)
    sinusoid = pos[None, :] / timescale[:, None]
    cos_h = np.cos(sinusoid).astype(np.float32)
    sin_h = np.sin(sinusoid).astype(np.float32)
    cosT = np.ascontiguousarray(np.concatenate([cos_h, cos_h], axis=0))
    sinST = np.ascontiguousarray(np.concatenate([-sin_h, sin_h], axis=0))

    if _NC_CACHE is None:
        _NC_CACHE = _build_bass()
    nc = _NC_CACHE

    in_maps = []
    for c in range(NCORES):
        in_maps.append({
            "xT": xT,
            "wq": np.ascontiguousarray(wq[c * NHC:(c + 1) * NHC]).astype(bf16),
            "wk": np.ascontiguousarray(wk[c]).astype(bf16),
            "wv": np.ascontiguousarray(wv[c]).astype(bf16),
            "wo": np.ascontiguousarray(wo[c * NHC:(c + 1) * NHC]).astype(bf16),
            "cosT": cosT,
            "sinST": sinST,
        })

    trace = os.environ.get("BASS_KERNEL_TRACE", "0") == "1"
    res = run_bass_kernel_spmd(nc, in_maps, list(range(NCORES)), trace=trace)
    global LAST_RESULTS
    LAST_RESULTS = res
    out = np.zeros((TOK, D), dtype=np.float32)
    for c in range(NCORES):
        out += res.results[c]["o_part"].astype(np.float32)
    return out.reshape(B, T, D)


LAST_RESULTS = None
